# revision 1
# baseline (speedup 1.0000x reference)
"""Causal self-attention Bass/Tile kernel for Trainium2, 8 NeuronCores (v3).

Sharding: batch (2) x head-groups (4 heads/core).  Host sums the 4 partial
output projections per batch.

Two-pass attention, no P transposes:
  pass 1 (stats): S[q,k] chunks in PSUM -> row max m (DVE), negated
  pass 2:        S^T[k,q] computed directly by a K=65 matmul where
                 row 64 of K^T is ones and row 64 of Q^T holds -m[q],
                 so PSUM already contains S^T - m.  ACT exp -> P^T fp16.
  diagonal causal masking via affine_select on P^T (zeros invalid k>q);
  AV matmul with [V | 1] fp16 appends the softmax denominator l as
  column 64; out rows scaled by 1/l during the PSUM->SBUF copy.
"""

import numpy as np

S = 2048
E = 1024
HPC = 4
D = 64
NCORES = 8
QB = S // 128     # 16 q-blocks
NSUP = S // 512   # 4 q-superblocks
KC = 8            # e chunks of 128
SCALE = 0.125     # 1/sqrt(64)

_CACHE = {}


def _build_nc():
    import concourse.bass as bass
    import concourse.mybir as mybir
    from concourse import tile

    f32 = mybir.dt.float32
    f16 = mybir.dt.float16
    X = mybir.AxisListType.X
    Exp = mybir.ActivationFunctionType.Exp

    nc = bass.Bass()

    x_d = nc.declare_dram_parameter("x", [S, E], f32, isOutput=False)
    wqk_d = nc.declare_dram_parameter("wqk", [E, HPC * 128], f32, isOutput=False)
    wv_d = nc.declare_dram_parameter("wv", [E, HPC * D], f32, isOutput=False)
    wo_d = nc.declare_dram_parameter("wo", [HPC * D, E], f32, isOutput=False)
    id_d = nc.declare_dram_parameter("ident", [128, 128], f32, isOutput=False)
    mask_d = nc.declare_dram_parameter("mask", [128, 128], f32, isOutput=False)
    out_d = nc.declare_dram_parameter("out", [S, E], f32, isOutput=True)

    with tile.TileContext(nc) as tc:
        with (
            tc.tile_pool(name="wpool", bufs=1) as wpool,
            tc.tile_pool(name="proj", bufs=1) as proj,
        ):
            # fused Q|K weights: wqk[:, c, h, 0:64] = W_q cols, [.., 64:128] = W_k
            wqk = wpool.tile([128, KC, HPC, 128], f32)
            wv = wpool.tile([128, KC, 256], f32)
            wo = wpool.tile([128, 2, E], f32)
            ident = wpool.tile([128, 128], f32)
            mask = wpool.tile([128, 128], f32)

            nc.sync.dma_start(
                wqk[:], wqk_d[:].rearrange("(c p) d -> p c d", p=128)
                .rearrange("p c (h e) -> p c h e", h=HPC))
            nc.sync.dma_start(wv[:], wv_d[:].rearrange("(c p) d -> p c d", p=128))
            nc.sync.dma_start(wo[:], wo_d[:].rearrange("(c p) e -> p c e", p=128))
            nc.sync.dma_start(ident[:], id_d[:])
            nc.sync.dma_start(mask[:], mask_d[:])

            # per-head QT/KT tiles [65, S]: rows 0-63 = projection (Q scaled),
            # row 64 of KT = ones, row 64 of QT = -m (written in phase C)
            qt = [proj.tile([65, S], f32, name=f"qt{h}", tag=f"qt{h}") for h in range(HPC)]
            kt = [proj.tile([65, S], f32, name=f"kt{h}", tag=f"kt{h}") for h in range(HPC)]
            vones = proj.tile([128, QB, HPC, D + 1], f16)

            # ---- phase A: x load + transpose (PE, via identity matmul) ----
            with tc.tile_pool(name="xtp", bufs=1) as xtp:
                xT = xtp.tile([128, KC, S], f32)
                with (
                    tc.tile_pool(name="xin", bufs=4) as xin,
                    tc.tile_pool(name="tps", bufs=2, space="PSUM") as tps,
                ):
                    for i in range(QB):
                        xt = xin.tile([128, E], f32)
                        nc.sync.dma_start(xt[:], x_d[128 * i:128 * (i + 1), :])
                        for g in range(2):
                            tp = tps.tile([128, 512], f32)
                            for t in range(4):
                                c = 4 * g + t
                                nc.tensor.matmul(
                                    tp[:, 128 * t:128 * (t + 1)],
                                    xt[:, 128 * c:128 * (c + 1)],
                                    ident[:],
                                )
                            dst = xT[:, 4 * g:4 * g + 4, 128 * i:128 * (i + 1)]
                            src = tp[:].rearrange("p (c s) -> p c s", c=4)
                            if g == 0:
                                nc.vector.tensor_copy(dst, src)
                            else:
                                nc.scalar.copy(dst, src)

                # ---- phase B: QKV projections ----
                with tc.tile_pool(name="qkv", bufs=4, space="PSUM") as qkv:
                    nc.vector.memset(vones[:, :, :, D:D + 1], 1.0)
                    for h in range(HPC):
                        nc.gpsimd.memset(kt[h][64:65, :], 1.0)
                    for h in range(HPC):
                        for sc in range(4):
                            pqk = qkv.tile([128, 512], f32, tag="qkv")
                            for c in range(KC):
                                nc.tensor.matmul(
                                    pqk[:], wqk[:, c, h, :],
                                    xT[:, c, 512 * sc:512 * (sc + 1)],
                                    start=(c == 0), stop=(c == KC - 1),
                                )
                            nc.scalar.mul(qt[h][0:D, 512 * sc:512 * (sc + 1)],
                                          pqk[0:D, :], SCALE)
                            nc.vector.tensor_copy(kt[h][0:D, 512 * sc:512 * (sc + 1)],
                                                  pqk[D:128, :])
                    for j in range(QB):
                        pv = qkv.tile([128, 256], f32, tag="qkv")
                        for c in range(KC):
                            nc.tensor.matmul(
                                pv[:], xT[:, c, 128 * j:128 * (j + 1)], wv[:, c, :],
                                start=(c == 0), stop=(c == KC - 1),
                            )
                        nc.scalar.copy(
                            vones[:, j, :, 0:D],
                            pv[:].rearrange("p (h d) -> p h d", h=HPC),
                        )

            # ---- phase C: attention ----
            attn = proj.tile([128, QB, HPC * D], f32)
            with (
                tc.tile_pool(name="sc", bufs=3, space="PSUM") as scp,
                tc.tile_pool(name="st", bufs=3, space="PSUM") as stp,
                tc.tile_pool(name="axp", bufs=2, space="PSUM") as axp,
                tc.tile_pool(name="pbuf", bufs=2) as pbuf,
                tc.tile_pool(name="stat", bufs=4) as stat,
            ):
                for h in range(HPC):
                    for sup in range(NSUP):
                        # --- pass 1: row-max stats for the 4 sub-blocks ---
                        negm4 = stat.tile([128, 4], f32, tag="n4")
                        for r in range(4):
                            i = 4 * sup + r
                            kn = 128 * (i + 1)
                            nch = (kn + 511) // 512
                            mparts = (stat.tile([128, 4], f32, tag="mp", name="mparts")
                                      if nch > 1 else None)
                            for c in range(nch):
                                n = min(512, kn - 512 * c)
                                sp = scp.tile([128, 512], f32, tag="sc")
                                nc.tensor.matmul(
                                    sp[:, :n],
                                    qt[h][0:D, 128 * i:128 * (i + 1)],
                                    kt[h][0:D, 512 * c:512 * c + n],
                                )
                                if c == nch - 1:
                                    # diagonal 128 cols: additive causal mask
                                    nc.vector.tensor_add(
                                        sp[:, n - 128:n], sp[:, n - 128:n], mask[:])
                                if nch == 1:
                                    nc.vector.reduce_max(negm4[:, r:r + 1], sp[:, :n],
                                                         axis=X, negate=True)
                                else:
                                    nc.vector.reduce_max(mparts[:, c:c + 1], sp[:, :n], axis=X)
                            if nch > 1:
                                nc.vector.reduce_max(negm4[:, r:r + 1], mparts[:, :nch],
                                                     axis=X, negate=True)
                        # --- transpose -m into qt row 64 ---
                        for r in range(4):
                            i = 4 * sup + r
                            nt = axp.tile([1, 128], f32, tag="ax", name="nt")
                            nc.tensor.matmul(nt[:], negm4[:, r:r + 1], ident[:])
                            dst = qt[h][64:65, 128 * i:128 * (i + 1)]
                            if r % 2 == 0:
                                nc.vector.tensor_copy(dst, nt[0:1, :])
                            else:
                                nc.scalar.copy(dst, nt[0:1, :])
                        # --- pass 2: S^T - m, exp -> P^T fp16 ---
                        pt = pbuf.tile([128, QB, 512], f16, tag="pt")
                        jmax = 4 * (sup + 1)
                        for j in range(jmax):
                            # causal: q-sub-blocks left of j's diagonal are
                            # invalid; skip them (stale pt there is zeroed by
                            # the affine_select below and never read by AV)
                            q0 = 128 * max(0, j - 4 * sup)
                            st = stp.tile([128, 512], f32, tag="st")
                            nc.tensor.matmul(
                                st[:, q0:],
                                kt[h][0:D + 1, 128 * j:128 * (j + 1)],
                                qt[h][0:D + 1, 512 * sup + q0:512 * (sup + 1)],
                            )
                            nc.scalar.activation(pt[:, j, q0:], st[:, q0:], Exp)
                        # --- causal mask on the 4 diagonal blocks ---
                        for r in range(4):
                            sl = pt[:, 4 * sup + r, 128 * r:]
                            nc.gpsimd.affine_select(
                                sl, sl,
                                pattern=[[1, 512 - 128 * r]],
                                compare_op=mybir.AluOpType.is_ge,
                                fill=0.0,
                                base=0,
                                channel_multiplier=-1,
                            )
                        # --- AV + denominator + normalize ---
                        for r in range(4):
                            i = 4 * sup + r
                            av = axp.tile([128, D + 1], f32, tag="ax")
                            for j in range(i + 1):
                                nc.tensor.matmul(
                                    av[:], pt[:, j, 128 * r:128 * (r + 1)],
                                    vones[:, j, h, :],
                                    start=(j == 0), stop=(j == i),
                                )
                            rl = stat.tile([128, 1], f32, tag="rl")
                            nc.vector.reciprocal(rl[:], av[:, D:D + 1])
                            nc.scalar.mul(attn[:, i, D * h:D * (h + 1)],
                                          av[:, 0:D], rl[:, 0:1])

            # ---- phase D: attn^T + output projection ----
            with (
                tc.tile_pool(name="aot", bufs=1) as aotp,
                tc.tile_pool(name="tps2", bufs=2, space="PSUM") as tps2,
                tc.tile_pool(name="ops", bufs=4, space="PSUM") as ops,
                tc.tile_pool(name="osb", bufs=3) as osb,
            ):
                aot = aotp.tile([128, 2, S], f32)
                for db in range(2):
                    for g in range(4):
                        tp = tps2.tile([128, 512], f32)
                        for t in range(4):
                            i = 4 * g + t
                            nc.tensor.matmul(
                                tp[:, 128 * t:128 * (t + 1)],
                                attn[:, i, 128 * db:128 * (db + 1)],
                                ident[:],
                            )
                        if g % 2 == 0:
                            nc.vector.tensor_copy(aot[:, db, 512 * g:512 * (g + 1)], tp[:])
                        else:
                            nc.scalar.copy(aot[:, db, 512 * g:512 * (g + 1)], tp[:])
                for sb in range(QB):
                    for ec in range(2):
                        po = ops.tile([128, 512], f32)
                        for kb in range(2):
                            nc.tensor.matmul(
                                po[:],
                                aot[:, kb, 128 * sb:128 * (sb + 1)],
                                wo[:, kb, 512 * ec:512 * (ec + 1)],
                                start=(kb == 0), stop=(kb == 1),
                            )
                        ob = osb.tile([128, 512], f32)
                        if ec == 0:
                            nc.scalar.copy(ob[:], po[:])
                        else:
                            nc.vector.tensor_copy(ob[:], po[:])
                        nc.sync.dma_start(
                            out_d[128 * sb:128 * (sb + 1), 512 * ec:512 * (ec + 1)],
                            ob[:],
                        )

    _split_excess_waits(nc)
    return nc


def _split_excess_waits(nc, maxw=1):
    """walrus here accepts one sync-wait per instruction; Tile's tail drain
    aggregates several.  Hoist excess waits onto preceding same-engine nops."""
    import concourse.mybir as mybir

    f = nc.m.functions[0]
    for b in f.blocks:
        insts = b.instructions
        i = 0
        while i < len(insts):
            inst = insts[i]
            si = inst.sync_info
            if si and si.on_wait and len(si.on_wait) > maxw:
                waits = list(si.on_wait)
                si.on_wait = waits[-maxw:]
                pos = i
                for w in waits[:-maxw]:
                    nop = nc.engines[inst.engine].nop(
                        nofuse=True, hint="wait_split"
                    ).ins
                    for bb in f.blocks:
                        L = bb.instructions
                        for k in range(len(L) - 1, -1, -1):
                            if L[k] is nop:
                                L.pop(k)
                                break
                    nsi = nop.sync_info
                    if nsi is None:
                        nop.sync_info = mybir.SyncInfo(on_wait=[w], on_update=[])
                    else:
                        nsi.on_wait = [w]
                    insts.insert(pos, nop)
                    pos += 1
                    i += 1
            i += 1


def _get_nc():
    if "nc" not in _CACHE:
        _CACHE["nc"] = _build_nc()
    return _CACHE["nc"]


def _make_in_maps(x, W_q, W_k, W_v, W_o):
    ident = np.eye(128, dtype=np.float32)
    r = np.arange(128)
    mask_np = np.where(r[None, :] <= r[:, None], 0.0, -1.0e30).astype(np.float32)
    in_maps = []
    for c in range(NCORES):
        b, g = c // 4, c % 4
        cs = slice(256 * g, 256 * (g + 1))
        wq_s = W_q[:, cs].reshape(E, HPC, D)
        wk_s = W_k[:, cs].reshape(E, HPC, D)
        wqk_s = np.concatenate([wq_s, wk_s], axis=2).reshape(E, HPC * 128)
        in_maps.append({
            "x": np.ascontiguousarray(x[b]),
            "wqk": np.ascontiguousarray(wqk_s),
            "mask": mask_np,
            "wv": np.ascontiguousarray(W_v[:, cs]),
            "wo": np.ascontiguousarray(W_o[cs, :]),
            "ident": ident,
        })
    return in_maps


def run_on_hw(x, W_q, W_k, W_v, W_o, trace=False):
    from concourse.bass_utils import run_bass_kernel_spmd

    nc = _get_nc()
    in_maps = _make_in_maps(x, W_q, W_k, W_v, W_o)
    res = run_bass_kernel_spmd(nc, in_maps, core_ids=list(range(NCORES)),
                               trace=trace)
    parts = [res.results[c]["out"] for c in range(NCORES)]
    out = np.stack([
        parts[0] + parts[1] + parts[2] + parts[3],
        parts[4] + parts[5] + parts[6] + parts[7],
    ]).astype(np.float32)
    return out, res


def kernel(x, W_q, W_k, W_v, W_o):
    x = np.asarray(x, dtype=np.float32)
    W_q = np.asarray(W_q, dtype=np.float32)
    W_k = np.asarray(W_k, dtype=np.float32)
    W_v = np.asarray(W_v, dtype=np.float32)
    W_o = np.asarray(W_o, dtype=np.float32)
    out, _ = run_on_hw(x, W_q, W_k, W_v, W_o, trace=False)
    return out



# revision 7
# speedup vs baseline: 1.9110x; 1.9110x over previous
"""Causal self-attention Bass/Tile kernel for Trainium2, 8 NeuronCores (v5).

Sharding: batch (2) x head-groups (4 heads/core).  Host sums the 4 partial
output projections per batch.

All tensor-engine work runs in fp16 (1 cyc/row vs 4 for fp32).  The Q/K
path needs ~16+ mantissa bits (scores are sums of ~1024 cancelling terms;
softmax near-argmax ties amplify per-element error), so it uses an exact
fp16 hi/lo split:
  - projections accumulate x_hi*w_hi + x_lo*w_hi + x_hi*w_lo (24 fp16
    chunk-matmuls) into one PSUM -> full-precision q;  q1 = fp16(q),
    q2 = fp16(q - q1)  => q1+q2 carries ~22 bits.
  - pass-2 computes S^T - m in TWO fp16 matmuls via partition stacking:
      [k1;k2] . [q1;q1]   (128-partition contraction)
    + [k1;ones] . [q2;-m] (65-partition contraction)
  - pass-1 (row max) runs on q1/k1 only: an O(1) error in m cancels in
    P/l and cannot overflow fp16.  The causal mask of the diagonal block
    is folded into the reduction via tensor_tensor_reduce.
V / AV / attn-transpose / out-projection are plain fp16 (loose budget).
x arrives pre-transposed and pre-split hi/lo from the host; partial
outputs return as fp16 and are summed on the host in fp32.

Phase C runs a 1-unit software-pipeline skew: AV of unit i-1 issues
between pass-1 and pass-2 of unit i, filling the PE while DVE reduces.
"""

import numpy as np

S = 2048
E = 1024
HPC = 4
D = 64
NCORES = 8
QB = S // 128     # 16 q-blocks
NSUP = S // 512   # 4 q-superblocks
KC = 8            # e chunks of 128
SCALE = 0.125     # 1/sqrt(64), folded into W_q host-side

_CACHE = {}


def _build_nc():
    import concourse.bass as bass
    import concourse.mybir as mybir
    from concourse import tile

    f32 = mybir.dt.float32
    f16 = mybir.dt.float16
    X = mybir.AxisListType.X
    Exp = mybir.ActivationFunctionType.Exp
    Add = mybir.AluOpType.add
    Max = mybir.AluOpType.max

    nc = bass.Bass()

    xthi_d = nc.declare_dram_parameter("xthi", [E, S], f16, isOutput=False)
    xtlo_d = nc.declare_dram_parameter("xtlo", [E, S], f16, isOutput=False)
    wqkhi_d = nc.declare_dram_parameter("wqkhi", [E, HPC * 128], f16, isOutput=False)
    wqklo_d = nc.declare_dram_parameter("wqklo", [E, HPC * 128], f16, isOutput=False)
    wv_d = nc.declare_dram_parameter("wv", [E, HPC * D], f16, isOutput=False)
    wo_d = nc.declare_dram_parameter("wo", [HPC * D, E], f16, isOutput=False)
    id_d = nc.declare_dram_parameter("ident", [128, 128], f16, isOutput=False)
    mask_d = nc.declare_dram_parameter("mask", [128, 128], f32, isOutput=False)
    out_d = nc.declare_dram_parameter("out", [S, E], f16, isOutput=True)

    with tile.TileContext(nc) as tc:
        with (
            tc.tile_pool(name="wpool", bufs=1) as wpool,
            tc.tile_pool(name="proj", bufs=1) as proj,
        ):
            wqkhi = wpool.tile([128, KC, HPC, 128], f16)
            wqklo = wpool.tile([128, KC, HPC, 128], f16)
            wv = wpool.tile([128, KC, HPC * D], f16)
            wo = wpool.tile([128, 2, E], f16)
            ident = wpool.tile([128, 128], f16)
            mask = wpool.tile([128, 128], f32)

            qt = [proj.tile([128, S], f16, name=f"qt{h}", tag=f"qt{h}") for h in range(HPC)]
            kt = [proj.tile([128, S], f16, name=f"kt{h}", tag=f"kt{h}") for h in range(HPC)]
            qlo = [proj.tile([65, S], f16, name=f"ql{h}", tag=f"ql{h}") for h in range(HPC)]
            kb = [proj.tile([65, S], f16, name=f"kb{h}", tag=f"kb{h}") for h in range(HPC)]
            vones = proj.tile([128, QB, HPC, D + 1], f16)
            attn = proj.tile([128, QB, HPC * D], f16)
            aot = proj.tile([128, 2, S], f16)

            # ---- phase B: QKV projections (x comes pre-transposed) ----
            with tc.tile_pool(name="xtp", bufs=1) as xtp:
                xthi = xtp.tile([128, KC, S], f16)
                xtlo = xtp.tile([128, KC, S], f16)

                # DMA: interleave so (sc=0, h=*) work starts earliest.
                # xt quarters on the SP queue, weights on the ACT queue.
                for c in range(KC):
                    nc.scalar.dma_start(
                        wqkhi[:, c], wqkhi_d[128 * c:128 * (c + 1), :]
                        .rearrange("p (h e) -> p h e", h=HPC))
                    nc.sync.dma_start(
                        xthi[:, c, 0:512], xthi_d[128 * c:128 * (c + 1), 0:512])
                for c in range(KC):
                    nc.sync.dma_start(
                        xtlo[:, c, 0:512], xtlo_d[128 * c:128 * (c + 1), 0:512])
                for c in range(KC):
                    nc.scalar.dma_start(
                        wqklo[:, c], wqklo_d[128 * c:128 * (c + 1), :]
                        .rearrange("p (h e) -> p h e", h=HPC))
                nc.scalar.dma_start(mask[:], mask_d[:])
                nc.scalar.dma_start(ident[:], id_d[:])
                for sc in range(1, 4):
                    cs = slice(512 * sc, 512 * (sc + 1))
                    nc.sync.dma_start(
                        xthi[:, :, cs],
                        xthi_d[:, cs].rearrange("(c p) s -> p c s", p=128))
                    nc.sync.dma_start(
                        xtlo[:, :, cs],
                        xtlo_d[:, cs].rearrange("(c p) s -> p c s", p=128))
                nc.scalar.dma_start(wv[:], wv_d[:].rearrange("(c p) d -> p c d", p=128))
                nc.scalar.dma_start(wo[:], wo_d[:].rearrange("(c p) e -> p c e", p=128))

                nc.vector.memset(vones[:, :, :, D:D + 1], 1.0)
                for h in range(HPC):
                    nc.gpsimd.memset(kb[h][64:65, :], 1.0)

                with tc.tile_pool(name="qkv", bufs=4, space="PSUM") as qkv:
                    for sc in range(4):
                        cs = slice(512 * sc, 512 * (sc + 1))
                        for h in range(HPC):
                            pqk = qkv.tile([128, 512], f32, tag="qkv")
                            for c in range(KC):
                                nc.tensor.matmul(
                                    pqk[:], wqkhi[:, c, h, :], xthi[:, c, cs],
                                    start=(c == 0), stop=False)
                            for c in range(KC):
                                nc.tensor.matmul(
                                    pqk[:], wqkhi[:, c, h, :], xtlo[:, c, cs],
                                    start=False, stop=False)
                            for c in range(KC):
                                nc.tensor.matmul(
                                    pqk[:], wqklo[:, c, h, :], xthi[:, c, cs],
                                    start=False, stop=(c == KC - 1))
                            # hi parts from PSUM (ACT), exact residuals (DVE),
                            # duplicates from SBUF (gpsimd)
                            nc.scalar.copy(qt[h][0:64, cs], pqk[0:64, :])
                            nc.scalar.copy(kt[h][0:64, cs], pqk[64:128, :])
                            nc.vector.tensor_sub(qlo[h][0:64, cs], pqk[0:64, :],
                                                 qt[h][0:64, cs])
                            nc.vector.tensor_sub(kt[h][64:128, cs], pqk[64:128, :],
                                                 kt[h][0:64, cs])
                            nc.gpsimd.tensor_copy(qt[h][64:128, cs], qt[h][0:64, cs])
                            nc.gpsimd.tensor_copy(kb[h][0:64, cs], kt[h][0:64, cs])
                    for j in range(QB):
                        pv = qkv.tile([128, 256], f32, tag="qkv")
                        for c in range(KC):
                            nc.tensor.matmul(
                                pv[:], xthi[:, c, 128 * j:128 * (j + 1)], wv[:, c, :],
                                start=(c == 0), stop=(c == KC - 1))
                        nc.scalar.copy(
                            vones[:, j, :, 0:D],
                            pv[:].rearrange("p (h d) -> p h d", h=HPC))

            # ---- phase C: attention (1-unit AV skew) + early phase-D T ----
            units = [(h, sup) for h in range(HPC) for sup in range(NSUP)]
            pts = {}

            with (
                tc.tile_pool(name="sc", bufs=3, space="PSUM") as scp,
                tc.tile_pool(name="st", bufs=3, space="PSUM") as stp,
                tc.tile_pool(name="axp", bufs=2, space="PSUM") as axp,
                tc.tile_pool(name="pbuf", bufs=2) as pbuf,
                tc.tile_pool(name="stat", bufs=4) as stat,
            ):
                def emit_pass1(h, sup):
                    negm4 = stat.tile([128, 4], f16, tag="n4")
                    for r in range(4):
                        i = 4 * sup + r
                        kn = 128 * (i + 1)
                        nch = (kn + 511) // 512
                        mparts = (stat.tile([128, 4], f32, tag="mp", name="mparts")
                                  if nch > 1 else None)
                        for c in range(nch):
                            n = min(512, kn - 512 * c)
                            sp = scp.tile([128, 512], f32, tag="sc")
                            nc.tensor.matmul(
                                sp[:, :n],
                                qt[h][0:D, 128 * i:128 * (i + 1)],
                                kt[h][0:D, 512 * c:512 * c + n])
                            if c == nch - 1:
                                # diagonal 128 cols: additive causal mask
                                nc.vector.tensor_add(
                                    sp[:, n - 128:n], sp[:, n - 128:n], mask[:])
                            if nch == 1:
                                nc.vector.reduce_max(negm4[:, r:r + 1], sp[:, :n],
                                                     axis=X, negate=True)
                            else:
                                nc.vector.reduce_max(mparts[:, c:c + 1], sp[:, :n],
                                                     axis=X)
                        if nch > 1:
                            nc.vector.reduce_max(negm4[:, r:r + 1], mparts[:, :nch],
                                                 axis=X, negate=True)
                    for r in range(4):
                        i = 4 * sup + r
                        nt = axp.tile([1, 128], f32, tag="ax", name="nt")
                        nc.tensor.matmul(nt[:], negm4[:, r:r + 1], ident[:])
                        dst = qlo[h][64:65, 128 * i:128 * (i + 1)]
                        if r % 2 == 0:
                            nc.vector.tensor_copy(dst, nt[0:1, :])
                        else:
                            nc.scalar.copy(dst, nt[0:1, :])

                def emit_pass2(h, sup, idx):
                    pt = pbuf.tile([128, QB, 512], f16, tag="pt")
                    pts[idx] = pt
                    jmax = 4 * (sup + 1)
                    for j in range(jmax):
                        q0 = 128 * max(0, j - 4 * sup)
                        st = stp.tile([128, 512], f32, tag="st")
                        nc.tensor.matmul(
                            st[:, q0:],
                            kt[h][:, 128 * j:128 * (j + 1)],
                            qt[h][:, 512 * sup + q0:512 * (sup + 1)],
                            start=True, stop=False)
                        nc.tensor.matmul(
                            st[:, q0:],
                            kb[h][0:65, 128 * j:128 * (j + 1)],
                            qlo[h][0:65, 512 * sup + q0:512 * (sup + 1)],
                            start=False, stop=True)
                        nc.scalar.activation(pt[:, j, q0:], st[:, q0:], Exp)
                    for r in range(4):
                        sl = pt[:, 4 * sup + r, 128 * r:]
                        nc.gpsimd.affine_select(
                            sl, sl,
                            pattern=[[1, 512 - 128 * r]],
                            compare_op=mybir.AluOpType.is_ge,
                            fill=0.0,
                            base=0,
                            channel_multiplier=-1,
                        )

                def emit_av(h, sup, idx):
                    pt = pts.pop(idx)
                    for r in range(4):
                        i = 4 * sup + r
                        av = axp.tile([128, D + 1], f32, tag="ax")
                        for j in range(i + 1):
                            nc.tensor.matmul(
                                av[:], pt[:, j, 128 * r:128 * (r + 1)],
                                vones[:, j, h, :],
                                start=(j == 0), stop=(j == i))
                        rl = stat.tile([128, 1], f32, tag="rl")
                        nc.vector.reciprocal(rl[:], av[:, D:D + 1])
                        nc.scalar.mul(attn[:, i, D * h:D * (h + 1)],
                                      av[:, 0:D], rl[:, 0:1])

                def emit_transposes(db):
                    for g in range(4):
                        tp = stp.tile([128, 512], f32, tag="st")
                        for t in range(4):
                            i = 4 * g + t
                            nc.tensor.matmul(
                                tp[:, 128 * t:128 * (t + 1)],
                                attn[:, i, 128 * db:128 * (db + 1)],
                                ident[:])
                        if g % 2 == 0:
                            nc.vector.tensor_copy(aot[:, db, 512 * g:512 * (g + 1)], tp[:])
                        else:
                            nc.scalar.copy(aot[:, db, 512 * g:512 * (g + 1)], tp[:])

                for idx in range(len(units) + 1):
                    if idx < len(units):
                        h, sup = units[idx]
                        emit_pass1(h, sup)
                    if idx >= 1:
                        h2, sup2 = units[idx - 1]
                        emit_av(h2, sup2, idx - 1)
                        if idx - 1 == 2 * NSUP - 1:
                            emit_transposes(0)
                        elif idx - 1 == 4 * NSUP - 1:
                            emit_transposes(1)
                    if idx < len(units):
                        h, sup = units[idx]
                        emit_pass2(h, sup, idx)

            # ---- phase D: output projection ----
            with (
                tc.tile_pool(name="ops", bufs=4, space="PSUM") as ops,
                tc.tile_pool(name="osb", bufs=4) as osb,
            ):
                for sb in range(QB):
                    for ec in range(2):
                        po = ops.tile([128, 512], f32)
                        for dbk in range(2):
                            nc.tensor.matmul(
                                po[:],
                                aot[:, dbk, 128 * sb:128 * (sb + 1)],
                                wo[:, dbk, 512 * ec:512 * (ec + 1)],
                                start=(dbk == 0), stop=(dbk == 1))
                        ob = osb.tile([128, 512], f16)
                        if ec == 0:
                            nc.scalar.copy(ob[:], po[:])
                        else:
                            nc.vector.tensor_copy(ob[:], po[:])
                        nc.sync.dma_start(
                            out_d[128 * sb:128 * (sb + 1), 512 * ec:512 * (ec + 1)],
                            ob[:])

    _split_excess_waits(nc)
    return nc


def _split_excess_waits(nc, maxw=1):
    """walrus here accepts one sync-wait per instruction; Tile's tail drain
    aggregates several.  Hoist excess waits onto preceding same-engine nops."""
    import concourse.mybir as mybir

    f = nc.m.functions[0]
    for b in f.blocks:
        insts = b.instructions
        i = 0
        while i < len(insts):
            inst = insts[i]
            si = inst.sync_info
            if si and si.on_wait and len(si.on_wait) > maxw:
                waits = list(si.on_wait)
                si.on_wait = waits[-maxw:]
                pos = i
                for w in waits[:-maxw]:
                    nop = nc.engines[inst.engine].nop(
                        nofuse=True, hint="wait_split"
                    ).ins
                    for bb in f.blocks:
                        L = bb.instructions
                        for k in range(len(L) - 1, -1, -1):
                            if L[k] is nop:
                                L.pop(k)
                                break
                    nsi = nop.sync_info
                    if nsi is None:
                        nop.sync_info = mybir.SyncInfo(on_wait=[w], on_update=[])
                    else:
                        nsi.on_wait = [w]
                    insts.insert(pos, nop)
                    pos += 1
                    i += 1
            i += 1


def _get_nc():
    if "nc" not in _CACHE:
        _CACHE["nc"] = _build_nc()
    return _CACHE["nc"]


def _make_in_maps(x, W_q, W_k, W_v, W_o):
    f16 = np.float16
    ident = np.eye(128, dtype=f16)
    r = np.arange(128)
    mask_np = np.where(r[None, :] <= r[:, None], 0.0, -1.0e30).astype(np.float32)
    in_maps = []
    for c in range(NCORES):
        b, g = c // 4, c % 4
        cs = slice(256 * g, 256 * (g + 1))
        xt = np.ascontiguousarray(x[b].T)              # [E, S] f32
        xthi = xt.astype(f16)
        xtlo = (xt - xthi.astype(np.float32)).astype(f16)
        wq_s = (W_q[:, cs] * SCALE).reshape(E, HPC, D)
        wk_s = W_k[:, cs].reshape(E, HPC, D)
        wqk = np.concatenate([wq_s, wk_s], axis=2).reshape(E, HPC * 128)
        wqkhi = wqk.astype(f16)
        wqklo = (wqk - wqkhi.astype(np.float32)).astype(f16)
        in_maps.append({
            "xthi": xthi,
            "xtlo": xtlo,
            "wqkhi": np.ascontiguousarray(wqkhi),
            "wqklo": np.ascontiguousarray(wqklo),
            "mask": mask_np,
            "wv": np.ascontiguousarray(W_v[:, cs]).astype(f16),
            "wo": np.ascontiguousarray(W_o[cs, :]).astype(f16),
            "ident": ident,
        })
    return in_maps


def run_on_hw(x, W_q, W_k, W_v, W_o, trace=False):
    from concourse.bass_utils import run_bass_kernel_spmd

    nc = _get_nc()
    in_maps = _make_in_maps(x, W_q, W_k, W_v, W_o)
    res = run_bass_kernel_spmd(nc, in_maps, core_ids=list(range(NCORES)),
                               trace=trace)
    parts = [res.results[c]["out"].astype(np.float32) for c in range(NCORES)]
    out = np.stack([
        parts[0] + parts[1] + parts[2] + parts[3],
        parts[4] + parts[5] + parts[6] + parts[7],
    ]).astype(np.float32)
    return out, res


def kernel(x, W_q, W_k, W_v, W_o):
    x = np.asarray(x, dtype=np.float32)
    W_q = np.asarray(W_q, dtype=np.float32)
    W_k = np.asarray(W_k, dtype=np.float32)
    W_v = np.asarray(W_v, dtype=np.float32)
    W_o = np.asarray(W_o, dtype=np.float32)
    out, _ = run_on_hw(x, W_q, W_k, W_v, W_o, trace=False)
    return out
